# revision 9
# baseline (speedup 1.0000x reference)
"""DGCNN (4-layer linear GCN) Trainium2 kernel, 8-core SPMD.

Strategy
--------
Nodes are sharded across 8 NeuronCores (12500 each).  All index-derived
structure (degree buckets, ELL slot layout, gather offsets) is computed on the
host at call time and baked into per-core input tensors; the single SPMD
program is shape-static across cores.

Math: with ds = 1/sqrt(deg), dinv = 1/deg, each GCN layer is
    v_l   = (ds * h_l) @ W_l                  (PE matmuls, node-major psum)
    agg_l = segment_sum(v_l[col], row)        (fp16 table all-gather + per-edge
                                               indirect-DMA gather into a
                                               degree-bucketed ELL layout +
                                               DVE tensor_reduce)
    out_l = ds * (agg_l + v_l) + b_l
Layer 4 (width 1) uses w = v3 @ W4 carried as a 33rd table column so
v4 = dinv * (aggw + w) + (b3@W4) * ds needs no transpose of out3.

The gathered tables are fp16; everything else fp32.
"""

import numpy as np
from contextlib import ExitStack

P = 128
NCORES = 8
BUCKET_LADDER = [8, 16, 24, 32, 40, 48, 56, 64, 80, 96, 112, 128]

_CACHE = {}


# ----------------------------------------------------------------------------
# host-side planning (pure index/layout work)
# ----------------------------------------------------------------------------
class Plan:
    pass


def make_plan(row, col, N, nsh, f_node):
    pl = Plan()
    E = row.shape[0]
    starts = np.searchsorted(row, np.arange(N + 1)).astype(np.int64)
    deg_in = np.diff(starts)  # in-degree per node (no self loop)
    assert deg_in.max() <= BUCKET_LADDER[-1], deg_in.max()

    # bucket index per node
    ladder = np.asarray(BUCKET_LADDER)
    bidx_all = np.searchsorted(ladder, deg_in, side="left")  # D >= deg

    # per-core, per-bucket counts -> global tile plan
    nb = len(ladder)
    counts = np.zeros((NCORES, nb), np.int64)
    for c in range(NCORES):
        counts[c] = np.bincount(bidx_all[c * nsh:(c + 1) * nsh], minlength=nb)
    T = np.ceil(counts / P).max(axis=0).astype(np.int64)  # tiles per bucket
    # ensure every core has at least one pad node (zero row target)
    cap = int(T.sum() * P)
    if (cap - nsh) == 0:
        T[np.nonzero(T)[0][0]] += 1
        cap = int(T.sum() * P)
    keep = np.nonzero(T)[0]
    pl.buckets = [(int(ladder[b]), int(T[b])) for b in keep]
    pl.NT = int(T.sum())
    pl.NODES_PAD = pl.NT * P
    nodeoff = np.concatenate([[0], np.cumsum(T[keep]) * P])  # node offsets per kept bucket
    slot_cnt = [d * t for d, t in pl.buckets]
    pl.SLOT_PP = int(sum(slot_cnt))
    slotoff = np.concatenate([[0], np.cumsum(slot_cnt)])
    pl.slotoff = slotoff
    pl.nodeoff = nodeoff
    pl.R = NCORES * pl.NODES_PAD

    # padded position per node (global)
    pos_g = np.empty(N, np.int64)
    perm = np.full((NCORES, pl.NODES_PAD), -1, np.int64)
    for c in range(NCORES):
        ids = np.arange(c * nsh, (c + 1) * nsh)
        b = bidx_all[ids]
        order = np.argsort(b, kind="stable")
        ids_o = ids[order]
        b_o = b[order]
        # position within bucket
        for ki, gb in enumerate(keep):
            sel = ids_o[b_o == gb]
            pp = nodeoff[ki] + np.arange(sel.shape[0])
            perm[c, pp] = sel
            pos_g[sel] = c * pl.NODES_PAD + pp
    pl.perm = perm

    # per-core arrays
    pl.deg = np.zeros((NCORES, pl.NODES_PAD), np.float32)
    pl.offs = np.empty((NCORES, P, pl.SLOT_PP), np.int32)
    pl.eslot = np.empty((NCORES, P, pl.SLOT_PP), np.int64)  # edge id or -1
    for c in range(NCORES):
        pm = perm[c]
        real = pm >= 0
        d = np.zeros(pl.NODES_PAD, np.int64)
        d[real] = deg_in[pm[real]]
        pl.deg[c][real] = d[real] + 1.0
        st = np.zeros(pl.NODES_PAD, np.int64)
        st[real] = starts[pm[real]]
        pad_target = c * pl.NODES_PAD + int(np.nonzero(~real)[0][0])
        for ki, (D, Tb) in enumerate(pl.buckets):
            npos = nodeoff[ki] + np.arange(Tb * P)      # padded node index
            dd = np.arange(D)
            e = st[npos][:, None] + dd[None, :]          # [Tb*P, D]
            valid = dd[None, :] < d[npos][:, None]
            e = np.where(valid, e, -1)
            # slot column j = slotoff[ki] + t*D + d ; node npos = off + t*128 + p
            e3 = e.reshape(Tb, P, D)                     # [t, p, d]
            ecols = np.swapaxes(e3, 0, 1).reshape(P, Tb * D)
            pl.eslot[c][:, slotoff[ki]:slotoff[ki + 1]] = ecols
            o = np.full((P, Tb * D), pad_target, np.int64)
            m = ecols >= 0
            o[m] = pos_g[col[ecols[m]]]
            pl.offs[c][:, slotoff[ki]:slotoff[ki + 1]] = o
    return pl


# ----------------------------------------------------------------------------
# device program
# ----------------------------------------------------------------------------
def build_program(pl, f_node):
    import concourse.bass as bass
    import concourse.mybir as mybir
    import concourse.tile as tile
    from concourse import bacc

    fp32 = mybir.dt.float32
    fp16 = mybir.dt.float16
    i32 = mybir.dt.int32
    NT, SLOT_PP, R, NP = pl.NT, pl.SLOT_PP, pl.R, pl.NODES_PAD
    CHP = 256  # max slots/partition per gather chunk

    nc = bacc.Bacc(None, target_bir_lowering=False, debug=False)

    # ---- I/O ----
    xT_in = nc.dram_tensor("xT", [P, NP], fp32, kind="ExternalInput")
    ea_in = nc.dram_tensor("ea", [P, SLOT_PP], fp32, kind="ExternalInput")
    offs_in = nc.dram_tensor("offs", [P, SLOT_PP], i32, kind="ExternalInput")
    ds_nm_in = nc.dram_tensor("ds_nm", [P, NT], fp32, kind="ExternalInput")
    dinv_nm_in = nc.dram_tensor("dinv_nm", [P, NT], fp32, kind="ExternalInput")
    Wx_in = nc.dram_tensor("Wx", [f_node, 32], fp32, kind="ExternalInput")
    w1e_in = nc.dram_tensor("w1e", [P, 32], fp32, kind="ExternalInput")  # replicated row f_node of W1
    W2_in = nc.dram_tensor("W2", [32, 32], fp32, kind="ExternalInput")
    W3_in = nc.dram_tensor("W3", [32, 32], fp32, kind="ExternalInput")
    W3T_in = nc.dram_tensor("W3T", [32, 32], fp32, kind="ExternalInput")
    W4_in = nc.dram_tensor("W4", [32, 1], fp32, kind="ExternalInput")
    b1_in = nc.dram_tensor("b1r", [P, 32], fp32, kind="ExternalInput")
    b2_in = nc.dram_tensor("b2r", [P, 32], fp32, kind="ExternalInput")
    b3_in = nc.dram_tensor("b3r", [P, 32], fp32, kind="ExternalInput")
    b3T_in = nc.dram_tensor("b3T", [32, P], fp32, kind="ExternalInput")  # b3 replicated cols
    b4_in = nc.dram_tensor("b4r", [P, 1], fp32, kind="ExternalInput")
    out_dev = nc.dram_tensor("out_dev", [NT, P, 98], fp32, kind="ExternalOutput")

    # ---- internal DRAM ----
    vloc = [nc.dram_tensor(f"vloc{l}", [NP, f], fp16) for l, f in ((1, 32), (2, 32), (3, 33), (4, 1))]
    tabs = [nc.dram_tensor(f"table{l}", [R, f], fp16) for l, f in ((1, 32), (2, 32), (3, 33), (4, 1))]
    groups = [list(range(NCORES))]

    add = mybir.AluOpType.add
    mult = mybir.AluOpType.mult

    with tile.TileContext(nc) as tc:
        with (
            tc.tile_pool(name="big", bufs=1) as big,      # xT / h2T shared slot
            tc.tile_pool(name="sb", bufs=1) as sb,        # persistents
            tc.tile_pool(name="val", bufs=2) as valp,
            tc.tile_pool(name="eap", bufs=1) as eap,     # gather double buffer
            tc.tile_pool(name="ps", bufs=2, space="PSUM") as psp,
            tc.tile_pool(name="pst", bufs=2, space="PSUM") as pst,
        ):
            # ---------------- phase 0: loads ----------------
            xT = big.tile([P, NP], fp32, tag="bigmat")
            nc.sync.dma_start(xT[:], xT_in[:])
            offs = sb.tile([P, SLOT_PP], i32)
            nc.sync.dma_start(offs[:], offs_in[:])
            ea = eap.tile([P, SLOT_PP], fp32)
            nc.sync.dma_start(ea[:], ea_in[:])
            ds_nm = sb.tile([P, NT], fp32)
            nc.sync.dma_start(ds_nm[:], ds_nm_in[:])
            dinv_nm = sb.tile([P, NT], fp32)
            nc.sync.dma_start(dinv_nm[:], dinv_nm_in[:])
            Wx = sb.tile([f_node, 32], fp32)
            nc.sync.dma_start(Wx[:], Wx_in[:])
            w1e = sb.tile([P, 32], fp32)
            nc.sync.dma_start(w1e[:], w1e_in[:])
            W2 = sb.tile([32, 32], fp32)
            nc.sync.dma_start(W2[:], W2_in[:])
            W3e = sb.tile([32, 33], fp32)
            nc.sync.dma_start(W3e[:, 0:32], W3_in[:])
            W3T = sb.tile([32, 32], fp32)
            nc.sync.dma_start(W3T[:], W3T_in[:])
            W4 = sb.tile([32, 1], fp32)
            nc.sync.dma_start(W4[:], W4_in[:])
            b1r = sb.tile([P, 32], fp32)
            nc.sync.dma_start(b1r[:], b1_in[:])
            b2r = sb.tile([P, 32], fp32)
            nc.sync.dma_start(b2r[:], b2_in[:])
            b3r = sb.tile([P, 32], fp32)
            nc.sync.dma_start(b3r[:], b3_in[:])
            b3T = sb.tile([32, P], fp32)
            nc.sync.dma_start(b3T[:], b3T_in[:])
            b4r = sb.tile([P, 1], fp32)
            nc.sync.dma_start(b4r[:], b4_in[:])

            # W3e col 32 = W3 @ W4 ; c4 = b3 @ W4 (replicated over partitions)
            ps_w = pst.tile([32, 1], fp32, tag="pswv")
            nc.tensor.matmul(ps_w[:], W3T[:], W4[:], start=True, stop=True)
            nc.vector.tensor_copy(W3e[:, 32:33], ps_w[:])
            ps_c4 = pst.tile([P, 1], fp32, tag="pswv")
            nc.tensor.matmul(ps_c4[:], b3T[:], W4[:], start=True, stop=True)
            c4 = sb.tile([P, 1], fp32)
            nc.vector.tensor_copy(c4[:], ps_c4[:])

            # x_edge = per-bucket reduce of ea slots
            xe = sb.tile([P, NT], fp32)
            for ki, (D, Tb) in enumerate(pl.buckets):
                so, to = pl.slotoff[ki], pl.nodeoff[ki] // P
                nc.vector.tensor_reduce(
                    out=xe[:, to:to + Tb],
                    in_=ea[:, so:so + Tb * D].rearrange("p (t d) -> p t d", d=D),
                    axis=mybir.AxisListType.X, op=add)
            nc.sync.dma_start(
                out_dev[:, :, 0:1].rearrange("t p o -> p t o"),
                xe[:, :].unsqueeze(2))

            # persistent buffers
            vsb = sb.tile([P, NT * 33], fp32)
            agg = sb.tile([P, NT * 33], fp32)
            s_nm = sb.tile([P, NT * 32], fp32)
            out_nm = sb.tile([P, NT * 32], fp32)
            v4 = sb.tile([P, NT], fp32)
            agg4 = sb.tile([P, NT], fp32)
            identity = sb.tile([P, P], fp32)
            from concourse.masks import make_identity
            make_identity(nc, identity[:])

            def v_matmul(lhs_big, lhs_parts, rhs, fw):
                """v[:, t*fw:(t+1)*fw] = (lhs chunk t).T @ rhs for all tiles."""
                per_bank = max(1, 512 // fw)
                t = 0
                while t < NT:
                    n = min(per_bank, NT - t)
                    ps = psp.tile([P, per_bank * fw], fp32, tag="vps")
                    for k in range(n):
                        nc.tensor.matmul(
                            ps[:, k * fw:(k + 1) * fw],
                            lhs_big[0:lhs_parts, (t + k) * P:(t + k + 1) * P],
                            rhs[:],
                            start=True, stop=True)
                    for k in range(n):
                        nc.vector.tensor_scalar(
                            out=vsb[:, (t + k) * fw:(t + k + 1) * fw],
                            in0=ps[:, k * fw:(k + 1) * fw],
                            scalar1=ds_nm[:, t + k:t + k + 1], scalar2=None, op0=mult)
                    t += n

            def gather_reduce(table_l, fw, dst, dstw, dst_off):
                """dst[:, t*dstw+dst_off ...] = ELL-reduce of gathered table rows."""
                for ki, (D, Tb) in enumerate(pl.buckets):
                    G = max(1, CHP // D)
                    t = 0
                    while t < Tb:
                        g = min(G, Tb - t)
                        toff = pl.nodeoff[ki] // P + t
                        so = pl.slotoff[ki] + t * D
                        nsl = g * D
                        val = valp.tile([P, CHP * 33], fp16, tag="val")
                        for j in range(nsl):
                            nc.gpsimd.indirect_dma_start(
                                out=val[:, j * fw:(j + 1) * fw],
                                out_offset=None,
                                in_=table_l[:, :],
                                in_offset=bass.IndirectOffsetOnAxis(
                                    ap=offs[:, so + j:so + j + 1], axis=0),
                            )
                        if dstw == 1:
                            o = dst[:, toff:toff + g].unsqueeze(2)
                        else:
                            o = dst[:, toff * dstw: (toff + g) * dstw].rearrange(
                                "p (g f) -> p g f", f=dstw)
                        nc.vector.tensor_reduce(
                            out=o,
                            in_=val[:, 0:nsl * fw].rearrange("p (g d f) -> p g f d", d=D, f=fw),
                            axis=mybir.AxisListType.X, op=add)
                        t += g

            def expand_nm(a):  # [P, NT] -> broadcast over 32 cols
                return a[:, :].unsqueeze(2).broadcast_to((P, NT, 32))

            def rep_b(b):  # [P, 32] -> broadcast over NT tiles
                return b[:, :].unsqueeze(1).broadcast_to((P, NT, 32))

            def as3(a, fw=32):  # [P, NT*fw] -> [P, NT, fw]
                return a[:, 0:NT * fw].rearrange("p (t f) -> p t f", f=fw)

            h2T = None
            for l in (1, 2, 3):
                fw = 33 if l == 3 else 32
                # ---- v = (ds*h) @ W ----
                if l == 1:
                    v_matmul(xT, f_node, Wx, fw)
                    # rank-1 x_edge term: v1 += (ds*xe) (x) w1row
                    dsxe = v4  # reuse as scratch [P, NT]
                    nc.vector.tensor_tensor(out=dsxe[:], in0=xe[:], in1=ds_nm[:], op=mult)
                    tmp = s_nm
                    for t in range(NT):
                        nc.vector.tensor_scalar(
                            out=tmp[:, t * 32:(t + 1) * 32], in0=w1e[:],
                            scalar1=dsxe[:, t:t + 1], scalar2=None, op0=mult)
                    nc.vector.tensor_tensor(
                        out=vsb[:, 0:NT * 32], in0=vsb[:, 0:NT * 32], in1=tmp[:, 0:NT * 32], op=add)
                else:
                    v_matmul(h2T, 32, W2 if l == 2 else W3e, fw)

                # ---- table write + allgather ----
                nc.gpsimd.dma_start(
                    vloc[l - 1][:, :].rearrange("(t p) f -> p t f", p=P),
                    as3(vsb, fw))
                nc.gpsimd.collective_compute(
                    "AllGather", mybir.AluOpType.bypass, replica_groups=groups,
                    ins=[vloc[l - 1][:, :]], outs=[tabs[l - 1][:, :]])

                # ---- gather + segmented reduce ----
                import os as _os
                if _os.environ.get("KDBG", "") == f"tab{l}":
                    tdump = valp.tile([P, CHP * 33], fp16, tag="val")
                    nc.sync.dma_start(
                        tdump[:, 0:NT * fw].rearrange("p (t f) -> p t f", f=fw),
                        tabs[l - 1][0:NP, :].rearrange("(t p) f -> p t f", p=P))
                    nc.vector.tensor_copy(as3(out_nm), tdump[:, 0:NT * fw].rearrange("p (t f) -> p t f", f=fw)[:, :, 0:32])
                gather_reduce(tabs[l - 1], fw, agg, fw, 0)

                # ---- epilogue ----
                skip_ep = _os.environ.get("KDBG", "") == f"tab{l}"
                if not skip_ep:
                  nc.vector.tensor_tensor(
                    out=as3(s_nm), in0=as3(agg, fw)[:, :, 0:32], in1=as3(vsb, fw)[:, :, 0:32], op=add)
                if not skip_ep:
                  nc.vector.tensor_tensor(
                    out=as3(out_nm), in0=as3(s_nm), in1=expand_nm(ds_nm), op=mult)
                  nc.vector.tensor_tensor(
                    out=as3(out_nm), in0=as3(out_nm),
                    in1=rep_b(b1r if l == 1 else (b2r if l == 2 else b3r)), op=add)
                import os
                dbg = os.environ.get("KDBG", "")
                src = as3(out_nm)
                if dbg == f"v{l}":
                    src = as3(vsb, fw)[:, :, 0:32]
                elif dbg == f"tab{l}":
                    src = as3(out_nm)
                elif dbg == f"agg{l}":
                    src = as3(agg, fw)[:, :, 0:32]
                nc.sync.dma_start(
                    out_dev[:, :, 1 + (l - 1) * 32: 1 + l * 32].rearrange("t p f -> p t f"),
                    src)
                if l < 3:
                    if h2T is None:
                        h2T = big.tile([32, NP], fp32, tag="bigmat")
                    for t in range(NT):
                        pt = pst.tile([32, P], fp32, tag="ptr")
                        nc.tensor.transpose(pt[:], out_nm[:, t * 32:(t + 1) * 32], identity[:])
                        if t % 2 == 0:
                            nc.scalar.copy(h2T[:, t * P:(t + 1) * P], pt[:])
                        else:
                            nc.vector.tensor_copy(h2T[:, t * P:(t + 1) * P], pt[:])
                else:
                    # v4 = dinv*(aggw + w) + c4*ds
                    aggw = as3(agg, 33)[:, :, 32]
                    wcol = as3(vsb, 33)[:, :, 32]
                    nc.vector.tensor_tensor(out=v4[:], in0=aggw, in1=wcol, op=add)
                    nc.vector.tensor_tensor(out=v4[:], in0=v4[:], in1=dinv_nm[:], op=mult)
                    nc.vector.tensor_scalar(
                        out=agg4[:], in0=ds_nm[:], scalar1=c4[:, 0:1], scalar2=None, op0=mult)
                    nc.vector.tensor_tensor(out=v4[:], in0=v4[:], in1=agg4[:], op=add)

            # ---------------- layer 4 ----------------
            nc.gpsimd.dma_start(
                vloc[3][:, :].rearrange("(t p) f -> p t f", p=P),
                v4[:, :].unsqueeze(2))
            nc.gpsimd.collective_compute(
                "AllGather", mybir.AluOpType.bypass, replica_groups=groups,
                ins=[vloc[3][:, :]], outs=[tabs[3][:, :]])
            gather_reduce(tabs[3], 1, agg4, 1, 0)
            nc.vector.tensor_tensor(out=agg4[:], in0=agg4[:], in1=v4[:], op=add)
            nc.vector.tensor_tensor(out=agg4[:], in0=agg4[:], in1=ds_nm[:], op=mult)
            nc.vector.tensor_tensor(
                out=agg4[:], in0=agg4[:],
                in1=b4r[:, 0:1].broadcast_to((P, NT)), op=add)
            nc.sync.dma_start(
                out_dev[:, :, 97:98].rearrange("t p o -> p t o"),
                agg4[:, :].unsqueeze(2))

    nc.finalize()
    return nc


# ----------------------------------------------------------------------------
# entry point
# ----------------------------------------------------------------------------
def _prepare(row, col, N, E, f_node):
    key = (int(N), int(E), int(row[0]), int(row[-1]), int(col[0]), int(col[-1]))
    if key in _CACHE:
        return _CACHE[key]
    nsh = N // NCORES
    pl = make_plan(np.asarray(row), np.asarray(col), N, nsh, f_node)
    nc = build_program(pl, f_node)
    _CACHE[key] = (pl, nc)
    return pl, nc


LAST_WALL_NS = None


def kernel(x, edge_attr, row, col, W1, b1, W2, b2, W3, b3, W4, b4):
    global LAST_WALL_NS
    x = np.asarray(x); edge_attr = np.asarray(edge_attr)
    row = np.asarray(row); col = np.asarray(col)
    N, f_node = x.shape
    E = row.shape[0]
    nsh = N // NCORES
    pl, nc = _prepare(row, col, N, E, f_node)
    NT, NP = pl.NT, pl.NODES_PAD

    W1 = np.asarray(W1); W2 = np.asarray(W2); W3 = np.asarray(W3); W4 = np.asarray(W4)
    b1 = np.asarray(b1); b2 = np.asarray(b2); b3 = np.asarray(b3); b4 = np.asarray(b4)
    ones = np.ones((P, 1), np.float32)
    in_maps = []
    for c in range(NCORES):
        pm = pl.perm[c]
        real = pm >= 0
        xT = np.zeros((P, NP), np.float32)
        xT[:, real] = x[pm[real]].T
        ea = np.zeros((P, pl.SLOT_PP), np.float32)
        m = pl.eslot[c] >= 0
        ea[m] = edge_attr[:, 0][pl.eslot[c][m]]
        deg = pl.deg[c]
        ds = np.zeros(NP, np.float32)
        dinv = np.zeros(NP, np.float32)
        ds[real] = 1.0 / np.sqrt(deg[real])
        dinv[real] = 1.0 / deg[real]
        in_maps.append(dict(
            xT=xT, ea=ea, offs=pl.offs[c],
            ds_nm=ds.reshape(NT, P).T.copy(),
            dinv_nm=dinv.reshape(NT, P).T.copy(),
            Wx=W1[:f_node].copy(), w1e=ones @ W1[f_node:f_node + 1],
            W2=W2, W3=W3, W3T=W3.T.copy(), W4=W4,
            b1r=ones @ b1.reshape(1, 32), b2r=ones @ b2.reshape(1, 32),
            b3r=ones @ b3.reshape(1, 32), b3T=b3.reshape(32, 1) @ np.ones((1, P), np.float32),
            b4r=np.full((P, 1), b4[0], np.float32),
        ))

    import time
    from concourse.bass_utils import run_bass_kernel_spmd
    t0 = time.perf_counter()
    res = run_bass_kernel_spmd(nc, in_maps, list(range(NCORES)))
    LAST_WALL_NS = (time.perf_counter() - t0) * 1e9

    out = np.empty((N, f_node + 98), np.float32)
    out[:, :f_node] = x
    for c in range(NCORES):
        dev = res.results[c]["out_dev"].reshape(NP, 98)
        pm = pl.perm[c]
        real = pm >= 0
        out[pm[real], f_node:] = dev[real]
    return out


# revision 10
# speedup vs baseline: 1.2433x; 1.2433x over previous
"""DGCNN (4-layer linear GCN) Trainium2 kernel, 8-core SPMD.

Strategy
--------
Nodes are sharded across 8 NeuronCores (12500 each).  All index-derived
structure (degree buckets, ELL slot layout, gather offsets) is computed on the
host at call time and baked into per-core input tensors; the single SPMD
program is shape-static across cores.

Math: with ds = 1/sqrt(deg), dinv = 1/deg, each GCN layer is
    v_l   = (ds * h_l) @ W_l                  (PE matmuls, node-major psum)
    agg_l = segment_sum(v_l[col], row)        (fp16 table all-gather + per-edge
                                               indirect-DMA gather into a
                                               degree-bucketed ELL layout +
                                               DVE tensor_reduce)
    out_l = ds * (agg_l + v_l) + b_l
Layer 4 (width 1) uses w = v3 @ W4 carried as a 33rd table column so
v4 = dinv * (aggw + w) + (b3@W4) * ds needs no transpose of out3.

The gathered tables are fp16; everything else fp32.
"""

import numpy as np
from contextlib import ExitStack

P = 128
NCORES = 8
BUCKET_LADDER = [8, 16, 24, 32, 40, 48, 56, 64, 80, 96, 112, 128]

_CACHE = {}


# ----------------------------------------------------------------------------
# host-side planning (pure index/layout work)
# ----------------------------------------------------------------------------
class Plan:
    pass


def make_plan(row, col, N, nsh, f_node):
    pl = Plan()
    E = row.shape[0]
    starts = np.searchsorted(row, np.arange(N + 1)).astype(np.int64)
    deg_in = np.diff(starts)  # in-degree per node (no self loop)
    assert deg_in.max() <= BUCKET_LADDER[-1], deg_in.max()

    # bucket index per node
    ladder = np.asarray(BUCKET_LADDER)
    bidx_all = np.searchsorted(ladder, deg_in, side="left")  # D >= deg

    # per-core, per-bucket counts -> global tile plan
    nb = len(ladder)
    counts = np.zeros((NCORES, nb), np.int64)
    for c in range(NCORES):
        counts[c] = np.bincount(bidx_all[c * nsh:(c + 1) * nsh], minlength=nb)
    T = np.ceil(counts / P).max(axis=0).astype(np.int64)  # tiles per bucket
    # ensure every core has at least one pad node (zero row target)
    cap = int(T.sum() * P)
    if (cap - nsh) == 0:
        T[np.nonzero(T)[0][0]] += 1
        cap = int(T.sum() * P)
    keep = np.nonzero(T)[0]
    pl.buckets = [(int(ladder[b]), int(T[b])) for b in keep]
    pl.NT = int(T.sum())
    pl.NODES_PAD = pl.NT * P
    nodeoff = np.concatenate([[0], np.cumsum(T[keep]) * P])  # node offsets per kept bucket
    slot_cnt = [d * t for d, t in pl.buckets]
    pl.SLOT_PP = int(sum(slot_cnt))
    slotoff = np.concatenate([[0], np.cumsum(slot_cnt)])
    pl.slotoff = slotoff
    pl.nodeoff = nodeoff
    pl.R = NCORES * pl.NODES_PAD

    # padded position per node (global)
    pos_g = np.empty(N, np.int64)
    perm = np.full((NCORES, pl.NODES_PAD), -1, np.int64)
    for c in range(NCORES):
        ids = np.arange(c * nsh, (c + 1) * nsh)
        b = bidx_all[ids]
        order = np.argsort(b, kind="stable")
        ids_o = ids[order]
        b_o = b[order]
        # position within bucket
        for ki, gb in enumerate(keep):
            sel = ids_o[b_o == gb]
            pp = nodeoff[ki] + np.arange(sel.shape[0])
            perm[c, pp] = sel
            pos_g[sel] = c * pl.NODES_PAD + pp
    pl.perm = perm

    # per-core arrays
    pl.deg = np.zeros((NCORES, pl.NODES_PAD), np.float32)
    pl.offs = np.empty((NCORES, P, pl.SLOT_PP), np.int32)
    pl.eslot = np.empty((NCORES, P, pl.SLOT_PP), np.int64)  # edge id or -1
    for c in range(NCORES):
        pm = perm[c]
        real = pm >= 0
        d = np.zeros(pl.NODES_PAD, np.int64)
        d[real] = deg_in[pm[real]]
        pl.deg[c][real] = d[real] + 1.0
        st = np.zeros(pl.NODES_PAD, np.int64)
        st[real] = starts[pm[real]]
        pad_target = c * pl.NODES_PAD + int(np.nonzero(~real)[0][0])
        for ki, (D, Tb) in enumerate(pl.buckets):
            npos = nodeoff[ki] + np.arange(Tb * P)      # padded node index
            dd = np.arange(D)
            e = st[npos][:, None] + dd[None, :]          # [Tb*P, D]
            valid = dd[None, :] < d[npos][:, None]
            e = np.where(valid, e, -1)
            # slot column j = slotoff[ki] + t*D + d ; node npos = off + t*128 + p
            e3 = e.reshape(Tb, P, D)                     # [t, p, d]
            ecols = np.swapaxes(e3, 0, 1).reshape(P, Tb * D)
            pl.eslot[c][:, slotoff[ki]:slotoff[ki + 1]] = ecols
            o = np.full((P, Tb * D), pad_target, np.int64)
            m = ecols >= 0
            o[m] = pos_g[col[ecols[m]]]
            pl.offs[c][:, slotoff[ki]:slotoff[ki + 1]] = o
    return pl


# ----------------------------------------------------------------------------
# device program
# ----------------------------------------------------------------------------
def build_program(pl, f_node):
    import concourse.bass as bass
    import concourse.mybir as mybir
    import concourse.tile as tile
    from concourse import bacc

    fp32 = mybir.dt.float32
    fp16 = mybir.dt.float16
    i32 = mybir.dt.int32
    NT, SLOT_PP, R, NP = pl.NT, pl.SLOT_PP, pl.R, pl.NODES_PAD
    CHP = 256  # max slots/partition per gather chunk

    nc = bacc.Bacc(None, target_bir_lowering=False, debug=False)

    # ---- I/O ----
    xT_in = nc.dram_tensor("xT", [P, NP], fp32, kind="ExternalInput")
    ea_in = nc.dram_tensor("ea", [P, SLOT_PP], fp32, kind="ExternalInput")
    offs_in = nc.dram_tensor("offs", [P, SLOT_PP], i32, kind="ExternalInput")
    ds_nm_in = nc.dram_tensor("ds_nm", [P, NT], fp32, kind="ExternalInput")
    dinv_nm_in = nc.dram_tensor("dinv_nm", [P, NT], fp32, kind="ExternalInput")
    Wx_in = nc.dram_tensor("Wx", [f_node, 32], fp32, kind="ExternalInput")
    w1e_in = nc.dram_tensor("w1e", [P, 32], fp32, kind="ExternalInput")  # replicated row f_node of W1
    W2_in = nc.dram_tensor("W2", [32, 32], fp32, kind="ExternalInput")
    W3_in = nc.dram_tensor("W3", [32, 32], fp32, kind="ExternalInput")
    W3T_in = nc.dram_tensor("W3T", [32, 32], fp32, kind="ExternalInput")
    W4_in = nc.dram_tensor("W4", [32, 1], fp32, kind="ExternalInput")
    b1_in = nc.dram_tensor("b1r", [P, 32], fp32, kind="ExternalInput")
    b2_in = nc.dram_tensor("b2r", [P, 32], fp32, kind="ExternalInput")
    b3_in = nc.dram_tensor("b3r", [P, 32], fp32, kind="ExternalInput")
    b3T_in = nc.dram_tensor("b3T", [32, P], fp32, kind="ExternalInput")  # b3 replicated cols
    b4_in = nc.dram_tensor("b4r", [P, 1], fp32, kind="ExternalInput")
    out_dev = nc.dram_tensor("out_dev", [NT, P, 98], fp32, kind="ExternalOutput")

    # ---- internal DRAM ----
    vloc = [nc.dram_tensor(f"vloc{l}", [NP, f], fp16) for l, f in ((1, 32), (2, 32), (3, 33), (4, 1))]
    tabs = [nc.dram_tensor(f"table{l}", [R, f], fp16) for l, f in ((1, 32), (2, 32), (3, 33), (4, 1))]
    groups = [list(range(NCORES))]

    add = mybir.AluOpType.add
    mult = mybir.AluOpType.mult

    with tile.TileContext(nc) as tc:
        with (
            tc.tile_pool(name="big", bufs=1) as big,      # xT / h2T shared slot
            tc.tile_pool(name="sb", bufs=1) as sb,        # persistents
            tc.tile_pool(name="val", bufs=2) as valp,
            tc.tile_pool(name="eap", bufs=1) as eap,     # gather double buffer
            tc.tile_pool(name="ps", bufs=2, space="PSUM") as psp,
            tc.tile_pool(name="pst", bufs=2, space="PSUM") as pst,
        ):
            # ---------------- phase 0: loads ----------------
            xT = big.tile([P, NP], fp32, tag="bigmat")
            nc.sync.dma_start(xT[:], xT_in[:])
            offs = sb.tile([P, SLOT_PP], i32)
            nc.sync.dma_start(offs[:], offs_in[:])
            ea = eap.tile([P, SLOT_PP], fp32)
            nc.sync.dma_start(ea[:], ea_in[:])
            ds_nm = sb.tile([P, NT], fp32)
            nc.sync.dma_start(ds_nm[:], ds_nm_in[:])
            dinv_nm = sb.tile([P, NT], fp32)
            nc.sync.dma_start(dinv_nm[:], dinv_nm_in[:])
            Wx = sb.tile([f_node, 32], fp32)
            nc.sync.dma_start(Wx[:], Wx_in[:])
            w1e = sb.tile([P, 32], fp32)
            nc.sync.dma_start(w1e[:], w1e_in[:])
            W2 = sb.tile([32, 32], fp32)
            nc.sync.dma_start(W2[:], W2_in[:])
            W3e = sb.tile([32, 33], fp32)
            nc.sync.dma_start(W3e[:, 0:32], W3_in[:])
            W3T = sb.tile([32, 32], fp32)
            nc.sync.dma_start(W3T[:], W3T_in[:])
            W4 = sb.tile([32, 1], fp32)
            nc.sync.dma_start(W4[:], W4_in[:])
            b1r = sb.tile([P, 32], fp32)
            nc.sync.dma_start(b1r[:], b1_in[:])
            b2r = sb.tile([P, 32], fp32)
            nc.sync.dma_start(b2r[:], b2_in[:])
            b3r = sb.tile([P, 32], fp32)
            nc.sync.dma_start(b3r[:], b3_in[:])
            b3T = sb.tile([32, P], fp32)
            nc.sync.dma_start(b3T[:], b3T_in[:])
            b4r = sb.tile([P, 1], fp32)
            nc.sync.dma_start(b4r[:], b4_in[:])

            # W3e col 32 = W3 @ W4 ; c4 = b3 @ W4 (replicated over partitions)
            ps_w = pst.tile([32, 1], fp32, tag="pswv")
            nc.tensor.matmul(ps_w[:], W3T[:], W4[:], start=True, stop=True)
            nc.vector.tensor_copy(W3e[:, 32:33], ps_w[:])
            ps_c4 = pst.tile([P, 1], fp32, tag="pswv")
            nc.tensor.matmul(ps_c4[:], b3T[:], W4[:], start=True, stop=True)
            c4 = sb.tile([P, 1], fp32)
            nc.vector.tensor_copy(c4[:], ps_c4[:])

            # x_edge = per-bucket reduce of ea slots
            xe = sb.tile([P, NT], fp32)
            for ki, (D, Tb) in enumerate(pl.buckets):
                so, to = pl.slotoff[ki], pl.nodeoff[ki] // P
                nc.vector.tensor_reduce(
                    out=xe[:, to:to + Tb],
                    in_=ea[:, so:so + Tb * D].rearrange("p (t d) -> p t d", d=D),
                    axis=mybir.AxisListType.X, op=add)
            nc.sync.dma_start(
                out_dev[:, :, 0:1].rearrange("t p o -> p t o"),
                xe[:, :].unsqueeze(2))

            # persistent buffers
            vsb = sb.tile([P, NT * 33], fp32)
            agg = sb.tile([P, NT * 33], fp32)
            s_nm = sb.tile([P, NT * 32], fp32)
            out_nm = sb.tile([P, NT * 32], fp32)
            v4 = sb.tile([P, NT], fp32)
            agg4 = sb.tile([P, NT], fp32)
            identity = sb.tile([P, P], fp32)
            from concourse.masks import make_identity
            make_identity(nc, identity[:])

            def v_matmul(lhs_big, lhs_parts, rhs, fw):
                """v[:, t*fw:(t+1)*fw] = (lhs chunk t).T @ rhs for all tiles."""
                per_bank = max(1, 512 // fw)
                t = 0
                while t < NT:
                    n = min(per_bank, NT - t)
                    ps = psp.tile([P, per_bank * fw], fp32, tag="vps")
                    for k in range(n):
                        nc.tensor.matmul(
                            ps[:, k * fw:(k + 1) * fw],
                            lhs_big[0:lhs_parts, (t + k) * P:(t + k + 1) * P],
                            rhs[:],
                            start=True, stop=True)
                    for k in range(n):
                        nc.vector.tensor_scalar(
                            out=vsb[:, (t + k) * fw:(t + k + 1) * fw],
                            in0=ps[:, k * fw:(k + 1) * fw],
                            scalar1=ds_nm[:, t + k:t + k + 1], scalar2=None, op0=mult)
                    t += n

            def gather_reduce(table_l, fw, dst, dstw, dst_off):
                """dst[:, t*dstw+dst_off ...] = ELL-reduce of gathered table rows."""
                for ki, (D, Tb) in enumerate(pl.buckets):
                    G = max(1, CHP // D)
                    t = 0
                    while t < Tb:
                        g = min(G, Tb - t)
                        toff = pl.nodeoff[ki] // P + t
                        so = pl.slotoff[ki] + t * D
                        nsl = g * D
                        val = valp.tile([P, CHP * 33], fp16, tag="val")
                        for j in range(nsl):
                            nc.gpsimd.indirect_dma_start(
                                out=val[:, j * fw:(j + 1) * fw],
                                out_offset=None,
                                in_=table_l[:, :],
                                in_offset=bass.IndirectOffsetOnAxis(
                                    ap=offs[:, so + j:so + j + 1], axis=0),
                            )
                        if dstw == 1:
                            o = dst[:, toff:toff + g].unsqueeze(2)
                        else:
                            o = dst[:, toff * dstw: (toff + g) * dstw].rearrange(
                                "p (g f) -> p g f", f=dstw)
                        nc.vector.tensor_reduce(
                            out=o,
                            in_=val[:, 0:nsl * fw].rearrange("p (g d f) -> p g f d", d=D, f=fw),
                            axis=mybir.AxisListType.X, op=add)
                        t += g

            def expand_nm(a):  # [P, NT] -> broadcast over 32 cols
                return a[:, :].unsqueeze(2).broadcast_to((P, NT, 32))

            def rep_b(b):  # [P, 32] -> broadcast over NT tiles
                return b[:, :].unsqueeze(1).broadcast_to((P, NT, 32))

            def as3(a, fw=32):  # [P, NT*fw] -> [P, NT, fw]
                return a[:, 0:NT * fw].rearrange("p (t f) -> p t f", f=fw)

            h2T = None
            for l in (1, 2, 3):
                fw = 33 if l == 3 else 32
                # ---- v = (ds*h) @ W ----
                if l == 1:
                    v_matmul(xT, f_node, Wx, fw)
                    # rank-1 x_edge term: v1 += (ds*xe) (x) w1row
                    dsxe = v4  # reuse as scratch [P, NT]
                    nc.vector.tensor_tensor(out=dsxe[:], in0=xe[:], in1=ds_nm[:], op=mult)
                    tmp = s_nm
                    for t in range(NT):
                        nc.vector.tensor_scalar(
                            out=tmp[:, t * 32:(t + 1) * 32], in0=w1e[:],
                            scalar1=dsxe[:, t:t + 1], scalar2=None, op0=mult)
                    nc.vector.tensor_tensor(
                        out=vsb[:, 0:NT * 32], in0=vsb[:, 0:NT * 32], in1=tmp[:, 0:NT * 32], op=add)
                else:
                    v_matmul(h2T, 32, W2 if l == 2 else W3e, fw)

                # ---- table write + allgather ----
                nc.gpsimd.dma_start(
                    vloc[l - 1][:, :].rearrange("(t p) f -> p t f", p=P),
                    as3(vsb, fw))
                nc.gpsimd.collective_compute(
                    "AllGather", mybir.AluOpType.bypass, replica_groups=groups,
                    ins=[vloc[l - 1][:, :]], outs=[tabs[l - 1][:, :]])

                # ---- gather + segmented reduce ----
                gather_reduce(tabs[l - 1], fw, agg, fw, 0)

                # ---- epilogue ----
                nc.vector.tensor_tensor(
                    out=as3(s_nm), in0=as3(agg, fw)[:, :, 0:32], in1=as3(vsb, fw)[:, :, 0:32], op=add)
                nc.vector.tensor_tensor(
                    out=as3(out_nm), in0=as3(s_nm), in1=expand_nm(ds_nm), op=mult)
                nc.vector.tensor_tensor(
                    out=as3(out_nm), in0=as3(out_nm),
                    in1=rep_b(b1r if l == 1 else (b2r if l == 2 else b3r)), op=add)
                nc.sync.dma_start(
                    out_dev[:, :, 1 + (l - 1) * 32: 1 + l * 32].rearrange("t p f -> p t f"),
                    as3(out_nm))
                if l < 3:
                    if h2T is None:
                        h2T = big.tile([32, NP], fp32, tag="bigmat")
                    for t in range(NT):
                        pt = pst.tile([32, P], fp32, tag="ptr")
                        nc.tensor.transpose(pt[:], out_nm[:, t * 32:(t + 1) * 32], identity[:])
                        if t % 2 == 0:
                            nc.scalar.copy(h2T[:, t * P:(t + 1) * P], pt[:])
                        else:
                            nc.vector.tensor_copy(h2T[:, t * P:(t + 1) * P], pt[:])
                else:
                    # v4 = dinv*(aggw + w) + c4*ds
                    aggw = as3(agg, 33)[:, :, 32]
                    wcol = as3(vsb, 33)[:, :, 32]
                    nc.vector.tensor_tensor(out=v4[:], in0=aggw, in1=wcol, op=add)
                    nc.vector.tensor_tensor(out=v4[:], in0=v4[:], in1=dinv_nm[:], op=mult)
                    nc.vector.tensor_scalar(
                        out=agg4[:], in0=ds_nm[:], scalar1=c4[:, 0:1], scalar2=None, op0=mult)
                    nc.vector.tensor_tensor(out=v4[:], in0=v4[:], in1=agg4[:], op=add)

            # ---------------- layer 4 ----------------
            nc.gpsimd.dma_start(
                vloc[3][:, :].rearrange("(t p) f -> p t f", p=P),
                v4[:, :].unsqueeze(2))
            nc.gpsimd.collective_compute(
                "AllGather", mybir.AluOpType.bypass, replica_groups=groups,
                ins=[vloc[3][:, :]], outs=[tabs[3][:, :]])
            gather_reduce(tabs[3], 1, agg4, 1, 0)
            nc.vector.tensor_tensor(out=agg4[:], in0=agg4[:], in1=v4[:], op=add)
            nc.vector.tensor_tensor(out=agg4[:], in0=agg4[:], in1=ds_nm[:], op=mult)
            nc.vector.tensor_tensor(
                out=agg4[:], in0=agg4[:],
                in1=b4r[:, 0:1].broadcast_to((P, NT)), op=add)
            nc.sync.dma_start(
                out_dev[:, :, 97:98].rearrange("t p o -> p t o"),
                agg4[:, :].unsqueeze(2))

    nc.finalize()
    return nc


# ----------------------------------------------------------------------------
# entry point
# ----------------------------------------------------------------------------
def _prepare(row, col, N, E, f_node):
    key = (int(N), int(E), int(row[0]), int(row[-1]), int(col[0]), int(col[-1]))
    if key in _CACHE:
        return _CACHE[key]
    nsh = N // NCORES
    pl = make_plan(np.asarray(row), np.asarray(col), N, nsh, f_node)
    nc = build_program(pl, f_node)
    _CACHE[key] = (pl, nc)
    return pl, nc


LAST_WALL_NS = None


def kernel(x, edge_attr, row, col, W1, b1, W2, b2, W3, b3, W4, b4):
    global LAST_WALL_NS
    x = np.asarray(x); edge_attr = np.asarray(edge_attr)
    row = np.asarray(row); col = np.asarray(col)
    N, f_node = x.shape
    E = row.shape[0]
    nsh = N // NCORES
    pl, nc = _prepare(row, col, N, E, f_node)
    NT, NP = pl.NT, pl.NODES_PAD

    W1 = np.asarray(W1); W2 = np.asarray(W2); W3 = np.asarray(W3); W4 = np.asarray(W4)
    b1 = np.asarray(b1); b2 = np.asarray(b2); b3 = np.asarray(b3); b4 = np.asarray(b4)
    ones = np.ones((P, 1), np.float32)
    in_maps = []
    for c in range(NCORES):
        pm = pl.perm[c]
        real = pm >= 0
        xT = np.zeros((P, NP), np.float32)
        xT[:, real] = x[pm[real]].T
        ea = np.zeros((P, pl.SLOT_PP), np.float32)
        m = pl.eslot[c] >= 0
        ea[m] = edge_attr[:, 0][pl.eslot[c][m]]
        deg = pl.deg[c]
        ds = np.zeros(NP, np.float32)
        dinv = np.zeros(NP, np.float32)
        ds[real] = 1.0 / np.sqrt(deg[real])
        dinv[real] = 1.0 / deg[real]
        in_maps.append(dict(
            xT=xT, ea=ea, offs=pl.offs[c],
            ds_nm=ds.reshape(NT, P).T.copy(),
            dinv_nm=dinv.reshape(NT, P).T.copy(),
            Wx=W1[:f_node].copy(), w1e=ones @ W1[f_node:f_node + 1],
            W2=W2, W3=W3, W3T=W3.T.copy(), W4=W4,
            b1r=ones @ b1.reshape(1, 32), b2r=ones @ b2.reshape(1, 32),
            b3r=ones @ b3.reshape(1, 32), b3T=b3.reshape(32, 1) @ np.ones((1, P), np.float32),
            b4r=np.full((P, 1), b4[0], np.float32),
        ))

    import time
    from concourse.bass_utils import run_bass_kernel_spmd
    t0 = time.perf_counter()
    res = run_bass_kernel_spmd(nc, in_maps, list(range(NCORES)))
    LAST_WALL_NS = (time.perf_counter() - t0) * 1e9

    out = np.empty((N, f_node + 98), np.float32)
    out[:, :f_node] = x
    for c in range(NCORES):
        dev = res.results[c]["out_dev"].reshape(NP, 98)
        pm = pl.perm[c]
        real = pm >= 0
        out[pm[real], f_node:] = dev[real]
    return out
